# revision 35
# baseline (speedup 1.0000x reference)
"""Trainium2 Bass kernel for the pairwise-score attention + gated MLP encoding.

Computation (per batch element b, p=1024 tokens, d=256 features):
    A[i,j]  = wa.P_i + wb.P_j + (P_i*wc).P_j
    itr     = softmax_j(A) @ P
    cat     = [P, itr]
    z       = tanh(cat@w1+b1); r = sigmoid(cat@w2+b2); f = sigmoid(cat@w3+b3)
    out     = r*P + f*z

Sharding: data-parallel over batch across 8 NeuronCores (4 batch el / core).

Kernel structure per batch element (PE path in bf16 so every matmul gets the
compiler-automatic Fast Weight Load; fp32r is FP32_HIGH mode which disables
FWL and makes each 128-col LDWEIGHTS ~184ns):
  - P DMA'd fp32, cast to bf16 on the scalar engine into [P | 1 1] (258-col)
    tiles; P^T via single-pass bf16 PE transposes (LDW-bound at ~2x the
    fp32r rate), pairs sharing one PSUM tile so DVE evacuations move 256 wide.
  - Scores transposed: S^T[j,i] = sum_d PT[d,j]*PcT[d,i].  The wa.P_i term
    cancels under softmax -> never computed.  The wb.P_j term is
    per-partition here -> folded into the exp as an ACT bias (sb = P.wb via
    Pool mul + DVE row-reduce on the exact fp32 P).
  - exp on the scalar engine from a 2-bank PSUM tile -> bf16 expST.
  - Attention fused with the softmax denominator: one PE stream per i-chunk
    with stationary = expST i-chunk and moving = [P | 1] (258 cols), so
    out[:, 0:256] is raw itr (natural layout) and out[:, 256] is the
    denominator -- a per-partition scalar, normalized by a [128,1] reciprocal
    + tensor_scalar multiply, then PE-transposed into the [d, i] layout the
    MLP consumes.  This replaces the separate all-ones denominator matmul
    (8192 PE columns/elem) with 16 cheap bf16 transposes.
  - MLP transposed (out^T = (cat@w)^T) so b1/b2/b3 are per-partition ACT
    biases; sigmoid evaluated as 0.5+0.5*tanh(x/2) so every activation stays
    in the one "exp_and_others" ACT table set.  Activations kept fp32.
  - Gating: m1 = (t2+1)*P^T, m2 = (t3+1)*z, o = m1+m2, all fp32
    scalar_tensor_tensor; the overall x0.5 is folded into the output
    transposes via a 0.5-scaled identity (fp32r, exact), which also removes
    the 0.5x evacuation of the output-transpose PSUM (tensor_scalar_mul),
    which also removes the baseline's separate Ph = P/2 tensor.
  - Emission is software-pipelined across batch elements: batch b+1's
    P^T-transposes and scores are emitted inside batch b's dependency bubbles
    so the (in-order) PE never idles long enough for HAM to re-throttle.
"""

import os
import sys

if "/opt/trn_rl_repo" not in sys.path:
    sys.path.insert(0, "/opt/trn_rl_repo")

import numpy as np

import concourse.bass as bass
import concourse.mybir as mybir
import concourse.tile as tile
from concourse import bacc
from concourse.bass_utils import run_bass_kernel_spmd
from concourse.masks import make_identity

F32 = mybir.dt.float32
F32R = mybir.dt.float32r
BF16 = mybir.dt.bfloat16
AF = mybir.ActivationFunctionType
ALU = mybir.AluOpType
AXX = mybir.AxisListType

B, PLEN, D = 32, 1024, 256
N_CORES = 8
B_LOC = B // N_CORES  # batch elements per core

NJ = PLEN // 128  # 8 token chunks of 128
ND = D // 128     # 2 feature chunks of 128


def _emit(ctx, tc, P_in, w_att, w_mlp, b_mlp, out):
    nc = tc.nc
    ts = bass.ts

    const = ctx.enter_context(tc.tile_pool(name="const", bufs=1))
    pin = ctx.enter_context(tc.tile_pool(name="pin", bufs=2))
    ptp = ctx.enter_context(tc.tile_pool(name="ptp", bufs=2))
    pexp = ctx.enter_context(tc.tile_pool(name="pexp", bufs=2))
    pitr = ctx.enter_context(tc.tile_pool(name="pitr", bufs=2))
    pmlp = ctx.enter_context(tc.tile_pool(name="pmlp", bufs=2))
    pout = ctx.enter_context(tc.tile_pool(name="pout", bufs=1))
    ps_big = ctx.enter_context(tc.tile_pool(name="ps_big", bufs=4, space="PSUM"))
    ps_t2 = ctx.enter_context(tc.tile_pool(name="ps_t2", bufs=2, space="PSUM"))
    ps_att = ctx.enter_context(tc.tile_pool(name="ps_att", bufs=2, space="PSUM"))

    # ---- constants (once per core) ----
    ident = const.tile([128, 128], F32)
    make_identity(nc, ident)
    ident_b = const.tile([128, 128], BF16)
    nc.vector.tensor_copy(out=ident_b, in_=ident)

    wc_sb = []
    for dc in range(ND):
        wc = const.tile([128, 1], F32, tag=f"wc{dc}")
        nc.gpsimd.dma_start(out=wc,
                            in_=w_att[bass.ds(2 * D + dc * 128, 128)].unsqueeze(1))
        wc_sb.append(wc)
    # wb as bf16 [128,1] columns: sb[j] = P_j.wb is computed on the PE as two
    # N=1 matmuls per token chunk riding the scores phase's PT stationaries
    wb_col = []
    for dc in range(ND):
        wf1 = const.tile([128, 1], F32, tag=f"wbf{dc}")
        nc.gpsimd.dma_start(out=wf1,
                            in_=w_att[bass.ds(D + dc * 128, 128)].unsqueeze(1))
        wb1 = const.tile([128, 1], BF16, tag=f"wbb{dc}")
        nc.vector.tensor_copy(out=wb1, in_=wf1)
        wb_col.append(wb1)

    # MLP weights: [512, 256] -> sbuf [128, 4(kc), 256] fp32 staging -> bf16
    w_sb = []
    for wi in range(3):
        wf = const.tile([128, 4, D], F32, tag=f"wf{wi}")
        nc.gpsimd.dma_start(
            out=wf, in_=w_mlp[wi].rearrange("(kc k) d -> k kc d", k=128))
        wt = const.tile([128, 4, D], BF16, tag=f"w{wi}")
        nc.vector.tensor_copy(out=wt, in_=wf)
        w_sb.append(wt)

    # biases, per dout-chunk [128,1]; for r/f (sigmoid-via-tanh) we need b/2
    b_sb = []  # b_sb[wi][dc]
    for wi in range(3):
        chunks = []
        for dc in range(ND):
            bt = const.tile([128, 1], F32, tag=f"b{wi}{dc}")
            nc.gpsimd.dma_start(out=bt,
                                in_=b_mlp[wi][bass.ds(dc * 128, 128)].unsqueeze(1))
            if wi > 0:
                bh = const.tile([128, 1], F32, tag=f"bh{wi}{dc}")
                nc.scalar.mul(out=bh, in_=bt, mul=0.5)
                bt = bh
            chunks.append(bt)
        b_sb.append(chunks)

    # ---- per-batch-element phases ----
    def phase_load(b, split=False):
        # per-chunk DMAs on the sync queue (DMA transfers occupy the issuing
        # engine's sequencer; sync is the only idle one).  ACT casts to bf16
        # [P | 1 1] tiles (258 cols even for ISA; col 256 is the ones column
        # that rides the itr matmul).
        Pf, Pn = [], []
        for jc in range(NJ):
            f = pin.tile([128, D], F32, tag=f"pf{jc}", name=f"pf{jc}")
            eng = nc.gpsimd if (split and jc % 2) else nc.sync
            eng.dma_start(out=f, in_=P_in[b, ts(jc, 128), :])
            t = pin.tile([128, D + 2], BF16, tag=f"pn{jc}", name=f"pn{jc}")
            nc.scalar.copy(out=t[:, 0:D], in_=f)
            nc.gpsimd.memset(t[:, D : D + 2], 1.0)
            Pf.append(f)
            Pn.append(t)
        return Pf, Pn

    def phase_pt(b, Pn):
        # P^T via paired single-pass bf16 PE transposes; PcT chunks emitted
        # right after each evacuation so the first scores matmul unblocks
        # after 2 of 4 j2 rounds instead of after the whole phase
        PT = [ptp.tile([128, PLEN], BF16, tag=f"pt{dc}", name=f"PT{dc}")
              for dc in range(ND)]
        PcT = [ptp.tile([128, PLEN], BF16, tag=f"pct{dc}", name=f"PcT{dc}")
               for dc in range(ND)]
        for j2 in range(NJ // 2):
            for dc in range(ND):
                pst = ps_t2.tile([128, 260], BF16, tag="pst", name="pst")
                nc.tensor.transpose(pst[:, 0:128], Pn[2 * j2][:, ts(dc, 128)],
                                    ident_b)
                nc.tensor.transpose(pst[:, 128:256], Pn[2 * j2 + 1][:, ts(dc, 128)],
                                    ident_b)
                nc.vector.tensor_copy(out=PT[dc][:, ts(j2, 256)],
                                      in_=pst[:, 0:256])
                nc.vector.tensor_scalar_mul(out=PcT[dc][:, ts(j2, 256)],
                                            in0=PT[dc][:, ts(j2, 256)],
                                            scalar1=wc_sb[dc])
        return PT, PcT

    def phase_scores(b, PT, PcT, Pn):
        # bias-free exp: exp(S + sb_j) = exp(S) * exp(sb_j), and since j is
        # the attn contraction partition, exp(sb_j) folds into a per-partition
        # scale of the [P | 1] moving operand (the denominator column scales
        # identically, so the softmax is unchanged).  sb[j] = sum_d PT[d,j]
        # wb[d] rides the scores stationaries as two 1-column matmuls per jc;
        # exp(sb) is read straight out of PSUM by the ACT.
        expST = [pexp.tile([128, PLEN], BF16, tag=f"es{jc}", name=f"expST{jc}")
                 for jc in range(NJ)]
        PnS = []
        sbp = None
        for jc in range(NJ):
            h = jc % 2
            if h == 0:
                sbt = ps_t2.tile([128, 260], BF16, tag="pst", name="psb")
                sbp = sbt[:, 256:260].bitcast(F32)
            for ic2 in range(2):
                pss = ps_big.tile([128, 512], F32, tag="big", name="pss")
                for dc in range(ND):
                    nc.tensor.matmul(pss, PT[dc][:, ts(jc, 128)],
                                     PcT[dc][:, ts(ic2, 512)],
                                     start=(dc == 0), stop=(dc == 1))
                nc.scalar.activation(out=expST[jc][:, ts(ic2, 512)], in_=pss,
                                     func=AF.Exp)
            for dc in range(ND):
                nc.tensor.matmul(sbp[:, h : h + 1], PT[dc][:, ts(jc, 128)],
                                 wb_col[dc], start=(dc == 0), stop=(dc == 1))
            if h == 1:
                es = pin.tile([128, 2], F32, tag=f"esb{jc // 2}",
                              name=f"esb{jc // 2}")
                nc.scalar.activation(out=es, in_=sbp, func=AF.Exp)
                for g in range(2):
                    t = pin.tile([128, D + 2], BF16, tag=f"pns{jc - 1 + g}",
                                 name=f"pns{jc - 1 + g}")
                    nc.gpsimd.tensor_scalar_mul(out=t, in0=Pn[jc - 1 + g],
                                                scalar1=es[:, g : g + 1])
                    PnS.append(t)
        return expST, PnS

    def phase_attn(b, PnS, expST):
        # itr + denominator in one PE stream per i-chunk: stationary is the
        # expST i-chunk, moving is [P | 1] (258 cols); output col 256 is the
        # softmax denominator for those 128 queries (per-partition scalar).
        itn = []
        for ic in range(NJ):
            pia = ps_att.tile([128, D + 2], F32, tag="pia", name=f"pia{ic}")
            for jc in range(NJ):
                nc.tensor.matmul(pia, expST[jc][:, ts(ic, 128)], PnS[jc],
                                 start=(jc == 0), stop=(jc == NJ - 1))
            recip = pitr.tile([128, 1], F32, tag="recip", name="recip", bufs=4)
            nc.vector.reciprocal_approx_fast(out=recip, in_=pia[:, D : D + 1])
            t = pitr.tile([128, D], BF16, tag=f"itn{ic}", name=f"itn{ic}")
            nc.vector.tensor_scalar_mul(out=t, in0=pia[:, 0:D], scalar1=recip)
            itn.append(t)
        # transpose normalized itr into the [d, i] layout the MLP consumes
        itrT = [pitr.tile([128, PLEN], BF16, tag=f"it{dc}", name=f"itrT{dc}")
                for dc in range(ND)]
        for dc in range(ND):
            for i2 in range(NJ // 2):
                pst = ps_t2.tile([128, 260], BF16, tag="pst", name="pst")
                nc.tensor.transpose(pst[:, 0:128], itn[2 * i2][:, ts(dc, 128)],
                                    ident_b)
                nc.tensor.transpose(pst[:, 128:256], itn[2 * i2 + 1][:, ts(dc, 128)],
                                    ident_b)
                nc.vector.tensor_copy(out=itrT[dc][:, ts(i2, 256)],
                                      in_=pst[:, 0:256])
        return itrT

    def phase_mlp(b, PT, itrT):
        catT = [PT[0], PT[1], itrT[0], itrT[1]]
        oT = []
        for dc in range(ND):
            acts = []
            for wi in range(3):
                t = pmlp.tile([128, PLEN], F32, tag=f"act{wi}", name=f"act{wi}")
                for pc in range(2):
                    psm = ps_big.tile([128, 512], F32, tag="big", name="psm")
                    for kc in range(4):
                        nc.tensor.matmul(
                            psm,
                            w_sb[wi][:, kc, ts(dc, 128)],
                            catT[kc][:, ts(pc, 512)],
                            start=(kc == 0), stop=(kc == 3),
                        )
                    nc.scalar.activation(out=t[:, ts(pc, 512)], in_=psm,
                                         func=AF.Tanh, bias=b_sb[wi][dc],
                                         scale=1.0 if wi == 0 else 0.5)
                acts.append(t)
            z_t, t2, t3 = acts
            # 2*out^T = (t2+1)*P^T + (t3+1)*z  (the x0.5 lives in the output
            # transpose identity), in p-halves so the output transposes can
            # start after the first half
            o = pmlp.tile([128, PLEN], BF16, tag=f"oT{dc}", name=f"oT{dc}")
            for pc in range(2):
                sl = ts(pc, 512)
                m1 = pmlp.tile([128, 512], F32, tag="m1", name="m1", bufs=1)
                nc.vector.scalar_tensor_tensor(out=m1, in0=t2[:, sl], scalar=1.0,
                                               in1=PT[dc][:, sl],
                                               op0=ALU.add, op1=ALU.mult)
                m2 = pmlp.tile([128, 512], F32, tag="m2", name="m2", bufs=1)
                nc.vector.scalar_tensor_tensor(out=m2, in0=t3[:, sl], scalar=1.0,
                                               in1=z_t[:, sl],
                                               op0=ALU.add, op1=ALU.mult)
                nc.vector.scalar_tensor_tensor(out=o[:, sl], in0=m2, scalar=0.0,
                                               in1=m1, op0=ALU.add, op1=ALU.add)
            oT.append(o)
        return oT

    def phase_mlp_out_tail(b, PT, itrT):
        # last batch element: no next-batch work exists to hide the
        # mlp->gating->transpose drain, so run it in p-halves -- the first
        # half's output transposes and DMAs overlap the second half's matmuls
        catT = [PT[0], PT[1], itrT[0], itrT[1]]
        oT = [pmlp.tile([128, PLEN], BF16, tag=f"oT{dc}", name=f"oT{dc}")
              for dc in range(ND)]
        for pc in range(2):
            sl = ts(pc, 512)
            for dc in range(ND):
                acts = []
                for wi in range(3):
                    psm = ps_big.tile([128, 512], F32, tag="big", name="psmh")
                    for kc in range(4):
                        nc.tensor.matmul(
                            psm,
                            w_sb[wi][:, kc, ts(dc, 128)],
                            catT[kc][:, sl],
                            start=(kc == 0), stop=(kc == 3),
                        )
                    t = pmlp.tile([128, 512], F32, tag=f"acth{wi}",
                                  name=f"acth{wi}")
                    nc.scalar.activation(out=t, in_=psm, func=AF.Tanh,
                                         bias=b_sb[wi][dc],
                                         scale=1.0 if wi == 0 else 0.5)
                    acts.append(t)
                z_t, t2, t3 = acts
                m1 = pmlp.tile([128, 512], F32, tag="m1", name="m1", bufs=1)
                nc.vector.scalar_tensor_tensor(out=m1, in0=t2, scalar=1.0,
                                               in1=PT[dc][:, sl],
                                               op0=ALU.add, op1=ALU.mult)
                m2 = pmlp.tile([128, 512], F32, tag="m2", name="m2", bufs=1)
                nc.vector.scalar_tensor_tensor(out=m2, in0=t3, scalar=1.0,
                                               in1=z_t,
                                               op0=ALU.add, op1=ALU.mult)
                nc.vector.scalar_tensor_tensor(out=oT[dc][:, sl], in0=m2,
                                               scalar=0.0, in1=m1,
                                               op0=ALU.add, op1=ALU.add)
        for pc in range(2):
            for p2 in range(pc * 4, pc * 4 + 4):
                onat = pout.tile([128, D], F32, tag=f"on{p2}", name=f"onat{p2}")
                pst = ps_t2.tile([128, 260], BF16, tag="pst", name="psto")
                nc.tensor.transpose(pst[:, 0:128], oT[0][:, ts(p2, 128)],
                                    ident_b)
                nc.tensor.transpose(pst[:, 128:256], oT[1][:, ts(p2, 128)],
                                    ident_b)
                if p2 % 2 == 0:
                    nc.scalar.mul(out=onat, in_=pst[:, 0:256], mul=0.5)
                else:
                    nc.vector.tensor_scalar_mul(out=onat, in0=pst[:, 0:256],
                                                scalar1=0.5)
                eng = [nc.sync, nc.gpsimd, nc.scalar][p2 % 3]
                eng.dma_start(out=out[b, ts(p2, 128), :], in_=onat)

    def phase_out(b, oT):
        # bf16 transposes (oT holds 2*out in bf16); the x0.5 rides the
        # fp32 evacuation, split across ACT/DVE to shorten the serial chain
        for p2 in range(NJ):
            onat = pout.tile([128, D], F32, tag=f"on{p2}", name=f"onat{p2}")
            pst = ps_t2.tile([128, 260], BF16, tag="pst", name="psto")
            nc.tensor.transpose(pst[:, 0:128], oT[0][:, ts(p2, 128)], ident_b)
            nc.tensor.transpose(pst[:, 128:256], oT[1][:, ts(p2, 128)], ident_b)
            if p2 % 2 == 0:
                nc.scalar.mul(out=onat, in_=pst[:, 0:256], mul=0.5)
            else:
                nc.vector.tensor_scalar_mul(out=onat, in0=pst[:, 0:256],
                                            scalar1=0.5)
            nc.sync.dma_start(out=out[b, ts(p2, 128), :], in_=onat)

    # ---- software-pipelined emission across batch elements ----
    # PE order per iteration: attn(b) | out(b-1) | pt(b+1) | mlp(b) |
    # scores(b+1) -- the out/pt phases fill the attn->mlp dependency bubble
    # so the (in-order) PE never idles long enough for HAM to re-throttle.
    Pf, Pn = phase_load(0, split=True)
    PT, PcT = phase_pt(0, Pn)
    expST, PnS = phase_scores(0, PT, PcT, Pn)
    oT_prev = None
    for b in range(B_LOC):
        last = b + 1 == B_LOC
        if not last:
            Pf_n, Pn_n = phase_load(b + 1)
        itrT = phase_attn(b, PnS, expST)
        if oT_prev is not None:
            phase_out(b - 1, oT_prev)
        if last:
            phase_mlp_out_tail(b, PT, itrT)
            break
        PT_n, PcT_n = phase_pt(b + 1, Pn_n)
        oT_prev = phase_mlp(b, PT, itrT)
        expST, PnS = phase_scores(b + 1, PT_n, PcT_n, Pn_n)
        Pn, PT, PcT = Pn_n, PT_n, PcT_n


_NC_CACHE = {}


def _build():
    if "nc" in _NC_CACHE:
        return _NC_CACHE["nc"]
    nc = bacc.Bacc("TRN2", target_bir_lowering=False, debug=False,
                   num_devices=N_CORES)
    P_in = nc.dram_tensor("p_in", [B_LOC, PLEN, D], F32, kind="ExternalInput").ap()
    w_att = nc.dram_tensor("w_att", [3 * D], F32, kind="ExternalInput").ap()
    w_mlp = [nc.dram_tensor(f"w{i}", [2 * D, D], F32, kind="ExternalInput").ap()
             for i in (1, 2, 3)]
    b_mlp = [nc.dram_tensor(f"b{i}", [D], F32, kind="ExternalInput").ap()
             for i in (1, 2, 3)]
    out = nc.dram_tensor("out", [B_LOC, PLEN, D], F32, kind="ExternalOutput").ap()

    from contextlib import ExitStack

    with tile.TileContext(nc) as tc, ExitStack() as ctx:
        _emit(ctx, tc, P_in, w_att, w_mlp, b_mlp, out)
    nc.compile()
    _NC_CACHE["nc"] = nc
    return nc


def run(inputs, trace=False, tmpdir=None):
    nc = _build()
    P = np.ascontiguousarray(np.asarray(inputs["P"], dtype=np.float32))
    shared = {
        "w_att": np.ascontiguousarray(np.asarray(inputs["w_itr_att"], np.float32)),
        "w1": np.ascontiguousarray(np.asarray(inputs["w1"], np.float32)),
        "w2": np.ascontiguousarray(np.asarray(inputs["w2"], np.float32)),
        "w3": np.ascontiguousarray(np.asarray(inputs["w3"], np.float32)),
        "b1": np.ascontiguousarray(np.asarray(inputs["b1"], np.float32)),
        "b2": np.ascontiguousarray(np.asarray(inputs["b2"], np.float32)),
        "b3": np.ascontiguousarray(np.asarray(inputs["b3"], np.float32)),
    }
    in_maps = [
        {"p_in": P[c * B_LOC : (c + 1) * B_LOC], **shared} for c in range(N_CORES)
    ]
    res = run_bass_kernel_spmd(nc, in_maps, list(range(N_CORES)), trace=trace,
                               tmpdir=tmpdir)
    full = np.concatenate([res.results[c]["out"] for c in range(N_CORES)], axis=0)
    return full, res


def kernel(**inputs):
    full, _ = run(inputs)
    return full


# revision 36
# speedup vs baseline: 2.0649x; 2.0649x over previous
"""Trainium2 Bass kernel for the pairwise-score attention + gated MLP encoding.

Computation (per batch element b, p=1024 tokens, d=256 features):
    A[i,j]  = wa.P_i + wb.P_j + (P_i*wc).P_j
    itr     = softmax_j(A) @ P
    cat     = [P, itr]
    z       = tanh(cat@w1+b1); r = sigmoid(cat@w2+b2); f = sigmoid(cat@w3+b3)
    out     = r*P + f*z

Sharding: data-parallel over batch across 8 NeuronCores (4 batch el / core).

Kernel structure per batch element (PE path in bf16 so every matmul gets the
compiler-automatic Fast Weight Load; fp32r is FP32_HIGH mode which disables
FWL and makes each 128-col LDWEIGHTS ~184ns):
  - P DMA'd fp32, cast to bf16 on the scalar engine into [P | 1 1] (258-col)
    tiles; P^T via single-pass bf16 PE transposes (LDW-bound at ~2x the
    fp32r rate), pairs sharing one PSUM tile so DVE evacuations move 256 wide.
  - Scores transposed: S^T[j,i] = sum_d PT[d,j]*PcT[d,i].  The wa.P_i term
    cancels under softmax -> never computed.  The wb.P_j term is
    per-partition here -> folded into the exp as an ACT bias (sb = P.wb via
    Pool mul + DVE row-reduce on the exact fp32 P).
  - exp on the scalar engine from a 2-bank PSUM tile -> bf16 expST.
  - Attention fused with the softmax denominator: one PE stream per i-chunk
    with stationary = expST i-chunk and moving = [P | 1] (258 cols), so
    out[:, 0:256] is raw itr (natural layout) and out[:, 256] is the
    denominator -- a per-partition scalar, normalized by a [128,1] reciprocal
    + tensor_scalar multiply, then PE-transposed into the [d, i] layout the
    MLP consumes.  This replaces the separate all-ones denominator matmul
    (8192 PE columns/elem) with 16 cheap bf16 transposes.
  - MLP transposed (out^T = (cat@w)^T) so b1/b2/b3 are per-partition ACT
    biases; sigmoid evaluated as 0.5+0.5*tanh(x/2) so every activation stays
    in the one "exp_and_others" ACT table set.  Activations kept fp32.
  - Gating: m1 = (t2+1)*P^T, m2 = (t3+1)*z, o = m1+m2, all fp32
    scalar_tensor_tensor; the overall x0.5 is folded into the output
    transposes via a 0.5-scaled identity (fp32r, exact), which also removes
    the 0.5x evacuation of the output-transpose PSUM (tensor_scalar_mul),
    which also removes the baseline's separate Ph = P/2 tensor.
  - Emission is software-pipelined across batch elements: batch b+1's
    P^T-transposes and scores are emitted inside batch b's dependency bubbles
    so the (in-order) PE never idles long enough for HAM to re-throttle.
"""

import os
import sys

if "/opt/trn_rl_repo" not in sys.path:
    sys.path.insert(0, "/opt/trn_rl_repo")

import numpy as np

import concourse.bass as bass
import concourse.mybir as mybir
import concourse.tile as tile
from concourse import bacc
from concourse.bass_utils import run_bass_kernel_spmd
from concourse.masks import make_identity

F32 = mybir.dt.float32
F32R = mybir.dt.float32r
BF16 = mybir.dt.bfloat16
AF = mybir.ActivationFunctionType
ALU = mybir.AluOpType
AXX = mybir.AxisListType

B, PLEN, D = 32, 1024, 256
N_CORES = 8
B_LOC = B // N_CORES  # batch elements per core

NJ = PLEN // 128  # 8 token chunks of 128
ND = D // 128     # 2 feature chunks of 128


def _emit(ctx, tc, P_in, w_att, w_mlp, b_mlp, out):
    nc = tc.nc
    ts = bass.ts

    const = ctx.enter_context(tc.tile_pool(name="const", bufs=1))
    pin = ctx.enter_context(tc.tile_pool(name="pin", bufs=2))
    ptp = ctx.enter_context(tc.tile_pool(name="ptp", bufs=2))
    pexp = ctx.enter_context(tc.tile_pool(name="pexp", bufs=2))
    pitr = ctx.enter_context(tc.tile_pool(name="pitr", bufs=2))
    pmlp = ctx.enter_context(tc.tile_pool(name="pmlp", bufs=2))
    pout = ctx.enter_context(tc.tile_pool(name="pout", bufs=1))
    ps_big = ctx.enter_context(tc.tile_pool(name="ps_big", bufs=4, space="PSUM"))
    ps_t2 = ctx.enter_context(tc.tile_pool(name="ps_t2", bufs=2, space="PSUM"))
    ps_att = ctx.enter_context(tc.tile_pool(name="ps_att", bufs=2, space="PSUM"))

    # ---- constants (once per core) ----
    ident = const.tile([128, 128], F32)
    make_identity(nc, ident)
    ident_b = const.tile([128, 128], BF16)
    nc.vector.tensor_copy(out=ident_b, in_=ident)

    wc_sb = []
    for dc in range(ND):
        wc = const.tile([128, 1], F32, tag=f"wc{dc}")
        nc.gpsimd.dma_start(out=wc,
                            in_=w_att[bass.ds(2 * D + dc * 128, 128)].unsqueeze(1))
        wc_sb.append(wc)
    # wb as bf16 [128,1] columns: sb[j] = P_j.wb is computed on the PE as two
    # N=1 matmuls per token chunk riding the scores phase's PT stationaries
    wb_col = []
    for dc in range(ND):
        wf1 = const.tile([128, 1], F32, tag=f"wbf{dc}")
        nc.gpsimd.dma_start(out=wf1,
                            in_=w_att[bass.ds(D + dc * 128, 128)].unsqueeze(1))
        wb1 = const.tile([128, 1], BF16, tag=f"wbb{dc}")
        nc.vector.tensor_copy(out=wb1, in_=wf1)
        wb_col.append(wb1)

    # MLP weights: [512, 256] -> sbuf [128, 4(kc), 256] fp32 staging -> bf16
    w_sb = []
    for wi in range(3):
        wf = const.tile([128, 4, D], F32, tag=f"wf{wi}")
        nc.gpsimd.dma_start(
            out=wf, in_=w_mlp[wi].rearrange("(kc k) d -> k kc d", k=128))
        wt = const.tile([128, 4, D], BF16, tag=f"w{wi}")
        nc.vector.tensor_copy(out=wt, in_=wf)
        w_sb.append(wt)

    # biases, per dout-chunk [128,1]; for r/f (sigmoid-via-tanh) we need b/2
    b_sb = []  # b_sb[wi][dc]
    for wi in range(3):
        chunks = []
        for dc in range(ND):
            bt = const.tile([128, 1], F32, tag=f"b{wi}{dc}")
            nc.gpsimd.dma_start(out=bt,
                                in_=b_mlp[wi][bass.ds(dc * 128, 128)].unsqueeze(1))
            if wi > 0:
                bh = const.tile([128, 1], F32, tag=f"bh{wi}{dc}")
                nc.scalar.mul(out=bh, in_=bt, mul=0.5)
                bt = bh
            chunks.append(bt)
        b_sb.append(chunks)

    # ---- per-batch-element phases ----
    def phase_load(b, split=False):
        # per-chunk DMAs on the sync queue (DMA transfers occupy the issuing
        # engine's sequencer; sync is the only idle one).  ACT casts to bf16
        # [P | 1 1] tiles (258 cols even for ISA; col 256 is the ones column
        # that rides the itr matmul).
        Pf, Pn = [], []
        for jc in range(NJ):
            f = pin.tile([128, D], F32, tag=f"pf{jc}", name=f"pf{jc}")
            eng = nc.gpsimd if (split and jc % 2) else nc.sync
            eng.dma_start(out=f, in_=P_in[b, ts(jc, 128), :])
            t = pin.tile([128, D + 2], BF16, tag=f"pn{jc}", name=f"pn{jc}")
            nc.scalar.copy(out=t[:, 0:D], in_=f)
            nc.gpsimd.memset(t[:, D : D + 2], 1.0)
            Pf.append(f)
            Pn.append(t)
        return Pf, Pn

    def phase_pt(b, Pn):
        # P^T via paired single-pass bf16 PE transposes; PcT chunks emitted
        # right after each evacuation so the first scores matmul unblocks
        # after 2 of 4 j2 rounds instead of after the whole phase
        PT = [ptp.tile([128, PLEN], BF16, tag=f"pt{dc}", name=f"PT{dc}")
              for dc in range(ND)]
        PcT = [ptp.tile([128, PLEN], BF16, tag=f"pct{dc}", name=f"PcT{dc}")
               for dc in range(ND)]
        for j2 in range(NJ // 2):
            for dc in range(ND):
                pst = ps_t2.tile([128, 260], BF16, tag="pst", name="pst")
                nc.tensor.transpose(pst[:, 0:128], Pn[2 * j2][:, ts(dc, 128)],
                                    ident_b)
                nc.tensor.transpose(pst[:, 128:256], Pn[2 * j2 + 1][:, ts(dc, 128)],
                                    ident_b)
                nc.vector.tensor_copy(out=PT[dc][:, ts(j2, 256)],
                                      in_=pst[:, 0:256])
                nc.vector.tensor_scalar_mul(out=PcT[dc][:, ts(j2, 256)],
                                            in0=PT[dc][:, ts(j2, 256)],
                                            scalar1=wc_sb[dc])
        return PT, PcT

    def phase_scores(b, PT, PcT, Pn):
        # bias-free exp: exp(S + sb_j) = exp(S) * exp(sb_j), and since j is
        # the attn contraction partition, exp(sb_j) folds into a per-partition
        # scale of the [P | 1] moving operand (the denominator column scales
        # identically, so the softmax is unchanged).  sb[j] = sum_d PT[d,j]
        # wb[d] rides the scores stationaries as two 1-column matmuls per jc;
        # exp(sb) is read straight out of PSUM by the ACT.
        expST = [pexp.tile([128, PLEN], BF16, tag=f"es{jc}", name=f"expST{jc}")
                 for jc in range(NJ)]
        PnS = []
        sbp = None
        for jc in range(NJ):
            h = jc % 2
            if h == 0:
                sbt = ps_t2.tile([128, 260], BF16, tag="pst", name="psb")
                sbp = sbt[:, 256:260].bitcast(F32)
            for ic2 in range(2):
                pss = ps_big.tile([128, 512], F32, tag="big", name="pss")
                for dc in range(ND):
                    nc.tensor.matmul(pss, PT[dc][:, ts(jc, 128)],
                                     PcT[dc][:, ts(ic2, 512)],
                                     start=(dc == 0), stop=(dc == 1))
                nc.scalar.activation(out=expST[jc][:, ts(ic2, 512)], in_=pss,
                                     func=AF.Exp)
            for dc in range(ND):
                nc.tensor.matmul(sbp[:, h : h + 1], PT[dc][:, ts(jc, 128)],
                                 wb_col[dc], start=(dc == 0), stop=(dc == 1))
            if h == 1:
                es = pin.tile([128, 2], F32, tag=f"esb{jc // 2}",
                              name=f"esb{jc // 2}")
                nc.scalar.activation(out=es, in_=sbp, func=AF.Exp)
                for g in range(2):
                    t = pin.tile([128, D + 2], BF16, tag=f"pns{jc - 1 + g}",
                                 name=f"pns{jc - 1 + g}")
                    nc.vector.tensor_scalar_mul(out=t, in0=Pn[jc - 1 + g],
                                                scalar1=es[:, g : g + 1])
                    PnS.append(t)
        return expST, PnS

    def phase_attn(b, PnS, expST):
        # itr + denominator in one PE stream per i-chunk: stationary is the
        # expST i-chunk, moving is [P | 1] (258 cols); output col 256 is the
        # softmax denominator for those 128 queries (per-partition scalar).
        itn = []
        for ic in range(NJ):
            pia = ps_att.tile([128, D + 2], F32, tag="pia", name=f"pia{ic}")
            for jc in range(NJ):
                nc.tensor.matmul(pia, expST[jc][:, ts(ic, 128)], PnS[jc],
                                 start=(jc == 0), stop=(jc == NJ - 1))
            recip = pitr.tile([128, 1], F32, tag="recip", name="recip", bufs=4)
            nc.vector.reciprocal_approx_fast(out=recip, in_=pia[:, D : D + 1])
            t = pitr.tile([128, D], BF16, tag=f"itn{ic}", name=f"itn{ic}")
            nc.vector.tensor_scalar_mul(out=t, in0=pia[:, 0:D], scalar1=recip)
            itn.append(t)
        # transpose normalized itr into the [d, i] layout the MLP consumes
        itrT = [pitr.tile([128, PLEN], BF16, tag=f"it{dc}", name=f"itrT{dc}")
                for dc in range(ND)]
        for dc in range(ND):
            for i2 in range(NJ // 2):
                pst = ps_t2.tile([128, 260], BF16, tag="pst", name="pst")
                nc.tensor.transpose(pst[:, 0:128], itn[2 * i2][:, ts(dc, 128)],
                                    ident_b)
                nc.tensor.transpose(pst[:, 128:256], itn[2 * i2 + 1][:, ts(dc, 128)],
                                    ident_b)
                nc.vector.tensor_copy(out=itrT[dc][:, ts(i2, 256)],
                                      in_=pst[:, 0:256])
        return itrT

    def phase_mlp(b, PT, itrT):
        catT = [PT[0], PT[1], itrT[0], itrT[1]]
        oT = []
        for dc in range(ND):
            acts = []
            for wi in range(3):
                t = pmlp.tile([128, PLEN], F32, tag=f"act{wi}", name=f"act{wi}")
                for pc in range(2):
                    psm = ps_big.tile([128, 512], F32, tag="big", name="psm")
                    for kc in range(4):
                        nc.tensor.matmul(
                            psm,
                            w_sb[wi][:, kc, ts(dc, 128)],
                            catT[kc][:, ts(pc, 512)],
                            start=(kc == 0), stop=(kc == 3),
                        )
                    nc.scalar.activation(out=t[:, ts(pc, 512)], in_=psm,
                                         func=AF.Tanh, bias=b_sb[wi][dc],
                                         scale=1.0 if wi == 0 else 0.5)
                acts.append(t)
            z_t, t2, t3 = acts
            # 2*out^T = (t2+1)*P^T + (t3+1)*z  (the x0.5 lives in the output
            # transpose identity), in p-halves so the output transposes can
            # start after the first half
            o = pmlp.tile([128, PLEN], BF16, tag=f"oT{dc}", name=f"oT{dc}")
            for pc in range(2):
                sl = ts(pc, 512)
                m1 = pmlp.tile([128, 512], F32, tag="m1", name="m1", bufs=1)
                nc.vector.scalar_tensor_tensor(out=m1, in0=t2[:, sl], scalar=1.0,
                                               in1=PT[dc][:, sl],
                                               op0=ALU.add, op1=ALU.mult)
                m2 = pmlp.tile([128, 512], F32, tag="m2", name="m2", bufs=1)
                nc.vector.scalar_tensor_tensor(out=m2, in0=t3[:, sl], scalar=1.0,
                                               in1=z_t[:, sl],
                                               op0=ALU.add, op1=ALU.mult)
                nc.vector.scalar_tensor_tensor(out=o[:, sl], in0=m2, scalar=0.0,
                                               in1=m1, op0=ALU.add, op1=ALU.add)
            oT.append(o)
        return oT

    def phase_mlp_out_tail(b, PT, itrT):
        # last batch element: no next-batch work exists to hide the
        # mlp->gating->transpose drain, so run it in p-halves -- the first
        # half's output transposes and DMAs overlap the second half's matmuls
        catT = [PT[0], PT[1], itrT[0], itrT[1]]
        oT = [pmlp.tile([128, PLEN], BF16, tag=f"oT{dc}", name=f"oT{dc}")
              for dc in range(ND)]
        for pc in range(2):
            sl = ts(pc, 512)
            for dc in range(ND):
                acts = []
                for wi in range(3):
                    psm = ps_big.tile([128, 512], F32, tag="big", name="psmh")
                    for kc in range(4):
                        nc.tensor.matmul(
                            psm,
                            w_sb[wi][:, kc, ts(dc, 128)],
                            catT[kc][:, sl],
                            start=(kc == 0), stop=(kc == 3),
                        )
                    t = pmlp.tile([128, 512], F32, tag=f"acth{wi}",
                                  name=f"acth{wi}")
                    nc.scalar.activation(out=t, in_=psm, func=AF.Tanh,
                                         bias=b_sb[wi][dc],
                                         scale=1.0 if wi == 0 else 0.5)
                    acts.append(t)
                z_t, t2, t3 = acts
                m1 = pmlp.tile([128, 512], F32, tag="m1", name="m1", bufs=1)
                nc.vector.scalar_tensor_tensor(out=m1, in0=t2, scalar=1.0,
                                               in1=PT[dc][:, sl],
                                               op0=ALU.add, op1=ALU.mult)
                m2 = pmlp.tile([128, 512], F32, tag="m2", name="m2", bufs=1)
                nc.vector.scalar_tensor_tensor(out=m2, in0=t3, scalar=1.0,
                                               in1=z_t,
                                               op0=ALU.add, op1=ALU.mult)
                nc.vector.scalar_tensor_tensor(out=oT[dc][:, sl], in0=m2,
                                               scalar=0.0, in1=m1,
                                               op0=ALU.add, op1=ALU.add)
        for pc in range(2):
            for p2 in range(pc * 4, pc * 4 + 4):
                onat = pout.tile([128, D], F32, tag=f"on{p2}", name=f"onat{p2}")
                pst = ps_t2.tile([128, 260], BF16, tag="pst", name="psto")
                nc.tensor.transpose(pst[:, 0:128], oT[0][:, ts(p2, 128)],
                                    ident_b)
                nc.tensor.transpose(pst[:, 128:256], oT[1][:, ts(p2, 128)],
                                    ident_b)
                if p2 % 2 == 0:
                    nc.scalar.mul(out=onat, in_=pst[:, 0:256], mul=0.5)
                else:
                    nc.vector.tensor_scalar_mul(out=onat, in0=pst[:, 0:256],
                                                scalar1=0.5)
                eng = [nc.sync, nc.gpsimd, nc.scalar][p2 % 3]
                eng.dma_start(out=out[b, ts(p2, 128), :], in_=onat)

    def phase_out(b, oT):
        # bf16 transposes (oT holds 2*out in bf16); the x0.5 rides the
        # fp32 evacuation, split across ACT/DVE to shorten the serial chain
        for p2 in range(NJ):
            onat = pout.tile([128, D], F32, tag=f"on{p2}", name=f"onat{p2}")
            pst = ps_t2.tile([128, 260], BF16, tag="pst", name="psto")
            nc.tensor.transpose(pst[:, 0:128], oT[0][:, ts(p2, 128)], ident_b)
            nc.tensor.transpose(pst[:, 128:256], oT[1][:, ts(p2, 128)], ident_b)
            if p2 % 2 == 0:
                nc.scalar.mul(out=onat, in_=pst[:, 0:256], mul=0.5)
            else:
                nc.vector.tensor_scalar_mul(out=onat, in0=pst[:, 0:256],
                                            scalar1=0.5)
            nc.sync.dma_start(out=out[b, ts(p2, 128), :], in_=onat)

    # ---- software-pipelined emission across batch elements ----
    # PE order per iteration: attn(b) | out(b-1) | pt(b+1) | mlp(b) |
    # scores(b+1) -- the out/pt phases fill the attn->mlp dependency bubble
    # so the (in-order) PE never idles long enough for HAM to re-throttle.
    Pf, Pn = phase_load(0, split=True)
    PT, PcT = phase_pt(0, Pn)
    expST, PnS = phase_scores(0, PT, PcT, Pn)
    oT_prev = None
    for b in range(B_LOC):
        last = b + 1 == B_LOC
        if not last:
            Pf_n, Pn_n = phase_load(b + 1)
        itrT = phase_attn(b, PnS, expST)
        if oT_prev is not None:
            phase_out(b - 1, oT_prev)
        if last:
            phase_mlp_out_tail(b, PT, itrT)
            break
        PT_n, PcT_n = phase_pt(b + 1, Pn_n)
        oT_prev = phase_mlp(b, PT, itrT)
        expST, PnS = phase_scores(b + 1, PT_n, PcT_n, Pn_n)
        Pn, PT, PcT = Pn_n, PT_n, PcT_n


_NC_CACHE = {}


def _build():
    if "nc" in _NC_CACHE:
        return _NC_CACHE["nc"]
    nc = bacc.Bacc("TRN2", target_bir_lowering=False, debug=False,
                   num_devices=N_CORES)
    P_in = nc.dram_tensor("p_in", [B_LOC, PLEN, D], F32, kind="ExternalInput").ap()
    w_att = nc.dram_tensor("w_att", [3 * D], F32, kind="ExternalInput").ap()
    w_mlp = [nc.dram_tensor(f"w{i}", [2 * D, D], F32, kind="ExternalInput").ap()
             for i in (1, 2, 3)]
    b_mlp = [nc.dram_tensor(f"b{i}", [D], F32, kind="ExternalInput").ap()
             for i in (1, 2, 3)]
    out = nc.dram_tensor("out", [B_LOC, PLEN, D], F32, kind="ExternalOutput").ap()

    from contextlib import ExitStack

    with tile.TileContext(nc) as tc, ExitStack() as ctx:
        _emit(ctx, tc, P_in, w_att, w_mlp, b_mlp, out)
    nc.compile()
    _NC_CACHE["nc"] = nc
    return nc


def run(inputs, trace=False, tmpdir=None):
    nc = _build()
    P = np.ascontiguousarray(np.asarray(inputs["P"], dtype=np.float32))
    shared = {
        "w_att": np.ascontiguousarray(np.asarray(inputs["w_itr_att"], np.float32)),
        "w1": np.ascontiguousarray(np.asarray(inputs["w1"], np.float32)),
        "w2": np.ascontiguousarray(np.asarray(inputs["w2"], np.float32)),
        "w3": np.ascontiguousarray(np.asarray(inputs["w3"], np.float32)),
        "b1": np.ascontiguousarray(np.asarray(inputs["b1"], np.float32)),
        "b2": np.ascontiguousarray(np.asarray(inputs["b2"], np.float32)),
        "b3": np.ascontiguousarray(np.asarray(inputs["b3"], np.float32)),
    }
    in_maps = [
        {"p_in": P[c * B_LOC : (c + 1) * B_LOC], **shared} for c in range(N_CORES)
    ]
    res = run_bass_kernel_spmd(nc, in_maps, list(range(N_CORES)), trace=trace,
                               tmpdir=tmpdir)
    full = np.concatenate([res.results[c]["out"] for c in range(N_CORES)], axis=0)
    return full, res


def kernel(**inputs):
    full, _ = run(inputs)
    return full


# revision 37
# speedup vs baseline: 2.0687x; 1.0019x over previous
"""Trainium2 Bass kernel for the pairwise-score attention + gated MLP encoding.

Computation (per batch element b, p=1024 tokens, d=256 features):
    A[i,j]  = wa.P_i + wb.P_j + (P_i*wc).P_j
    itr     = softmax_j(A) @ P
    cat     = [P, itr]
    z       = tanh(cat@w1+b1); r = sigmoid(cat@w2+b2); f = sigmoid(cat@w3+b3)
    out     = r*P + f*z

Sharding: data-parallel over batch across 8 NeuronCores (4 batch el / core).

Kernel structure per batch element (PE path in bf16 so every matmul gets the
compiler-automatic Fast Weight Load; fp32r is FP32_HIGH mode which disables
FWL and makes each 128-col LDWEIGHTS ~184ns):
  - P DMA'd fp32, cast to bf16 on the scalar engine into [P | 1 1] (258-col)
    tiles; P^T via single-pass bf16 PE transposes (LDW-bound at ~2x the
    fp32r rate), pairs sharing one PSUM tile so DVE evacuations move 256 wide.
  - Scores transposed: S^T[j,i] = sum_d PT[d,j]*PcT[d,i].  The wa.P_i term
    cancels under softmax -> never computed.  The wb.P_j term is
    per-partition here -> folded into the exp as an ACT bias (sb = P.wb via
    Pool mul + DVE row-reduce on the exact fp32 P).
  - exp on the scalar engine from a 2-bank PSUM tile -> bf16 expST.
  - Attention fused with the softmax denominator: one PE stream per i-chunk
    with stationary = expST i-chunk and moving = [P | 1] (258 cols), so
    out[:, 0:256] is raw itr (natural layout) and out[:, 256] is the
    denominator -- a per-partition scalar, normalized by a [128,1] reciprocal
    + tensor_scalar multiply, then PE-transposed into the [d, i] layout the
    MLP consumes.  This replaces the separate all-ones denominator matmul
    (8192 PE columns/elem) with 16 cheap bf16 transposes.
  - MLP transposed (out^T = (cat@w)^T) so b1/b2/b3 are per-partition ACT
    biases; sigmoid evaluated as 0.5+0.5*tanh(x/2) so every activation stays
    in the one "exp_and_others" ACT table set.  Activations kept fp32.
  - Gating: m1 = (t2+1)*P^T, m2 = (t3+1)*z, o = m1+m2, all fp32
    scalar_tensor_tensor; the overall x0.5 is folded into the output
    transposes via a 0.5-scaled identity (fp32r, exact), which also removes
    the 0.5x evacuation of the output-transpose PSUM (tensor_scalar_mul),
    which also removes the baseline's separate Ph = P/2 tensor.
  - Emission is software-pipelined across batch elements: batch b+1's
    P^T-transposes and scores are emitted inside batch b's dependency bubbles
    so the (in-order) PE never idles long enough for HAM to re-throttle.
"""

import os
import sys

if "/opt/trn_rl_repo" not in sys.path:
    sys.path.insert(0, "/opt/trn_rl_repo")

import numpy as np

import concourse.bass as bass
import concourse.mybir as mybir
import concourse.tile as tile
from concourse import bacc
from concourse.bass_utils import run_bass_kernel_spmd
from concourse.masks import make_identity

F32 = mybir.dt.float32
F32R = mybir.dt.float32r
BF16 = mybir.dt.bfloat16
AF = mybir.ActivationFunctionType
ALU = mybir.AluOpType
AXX = mybir.AxisListType

B, PLEN, D = 32, 1024, 256
N_CORES = 8
B_LOC = B // N_CORES  # batch elements per core

NJ = PLEN // 128  # 8 token chunks of 128
ND = D // 128     # 2 feature chunks of 128


def _emit(ctx, tc, P_in, w_att, w_mlp, b_mlp, out):
    nc = tc.nc
    ts = bass.ts

    const = ctx.enter_context(tc.tile_pool(name="const", bufs=1))
    pin = ctx.enter_context(tc.tile_pool(name="pin", bufs=2))
    ptp = ctx.enter_context(tc.tile_pool(name="ptp", bufs=2))
    pexp = ctx.enter_context(tc.tile_pool(name="pexp", bufs=2))
    pitr = ctx.enter_context(tc.tile_pool(name="pitr", bufs=2))
    pmlp = ctx.enter_context(tc.tile_pool(name="pmlp", bufs=2))
    pout = ctx.enter_context(tc.tile_pool(name="pout", bufs=1))
    ps_big = ctx.enter_context(tc.tile_pool(name="ps_big", bufs=4, space="PSUM"))
    ps_t2 = ctx.enter_context(tc.tile_pool(name="ps_t2", bufs=2, space="PSUM"))
    ps_att = ctx.enter_context(tc.tile_pool(name="ps_att", bufs=2, space="PSUM"))

    # ---- constants (once per core) ----
    ident = const.tile([128, 128], F32)
    make_identity(nc, ident)
    ident_b = const.tile([128, 128], BF16)
    nc.vector.tensor_copy(out=ident_b, in_=ident)

    wc_sb = []
    for dc in range(ND):
        wc = const.tile([128, 1], F32, tag=f"wc{dc}")
        nc.gpsimd.dma_start(out=wc,
                            in_=w_att[bass.ds(2 * D + dc * 128, 128)].unsqueeze(1))
        wc_sb.append(wc)
    # wb as bf16 [128,1] columns: sb[j] = P_j.wb is computed on the PE as two
    # N=1 matmuls per token chunk riding the scores phase's PT stationaries
    wb_col = []
    for dc in range(ND):
        wf1 = const.tile([128, 1], F32, tag=f"wbf{dc}")
        nc.gpsimd.dma_start(out=wf1,
                            in_=w_att[bass.ds(D + dc * 128, 128)].unsqueeze(1))
        wb1 = const.tile([128, 1], BF16, tag=f"wbb{dc}")
        nc.vector.tensor_copy(out=wb1, in_=wf1)
        wb_col.append(wb1)

    # MLP weights: [512, 256] -> sbuf [128, 4(kc), 256] fp32 staging -> bf16
    w_sb = []
    for wi in range(3):
        wf = const.tile([128, 4, D], F32, tag=f"wf{wi}")
        nc.gpsimd.dma_start(
            out=wf, in_=w_mlp[wi].rearrange("(kc k) d -> k kc d", k=128))
        wt = const.tile([128, 4, D], BF16, tag=f"w{wi}")
        nc.vector.tensor_copy(out=wt, in_=wf)
        w_sb.append(wt)

    # biases, per dout-chunk [128,1]; for r/f (sigmoid-via-tanh) we need b/2
    b_sb = []  # b_sb[wi][dc]
    for wi in range(3):
        chunks = []
        for dc in range(ND):
            bt = const.tile([128, 1], F32, tag=f"b{wi}{dc}")
            nc.gpsimd.dma_start(out=bt,
                                in_=b_mlp[wi][bass.ds(dc * 128, 128)].unsqueeze(1))
            if wi > 0:
                bh = const.tile([128, 1], F32, tag=f"bh{wi}{dc}")
                nc.scalar.mul(out=bh, in_=bt, mul=0.5)
                bt = bh
            chunks.append(bt)
        b_sb.append(chunks)

    # ---- per-batch-element phases ----
    def phase_load(b, split=False):
        # per-chunk DMAs on the sync queue (DMA transfers occupy the issuing
        # engine's sequencer; sync is the only idle one).  ACT casts to bf16
        # [P | 1 1] tiles (258 cols even for ISA; col 256 is the ones column
        # that rides the itr matmul).
        Pf, Pn = [], []
        for jc in range(NJ):
            f = pin.tile([128, D], F32, tag=f"pf{jc}", name=f"pf{jc}")
            eng = nc.gpsimd if (split and jc % 2) else nc.sync
            eng.dma_start(out=f, in_=P_in[b, ts(jc, 128), :])
            t = pin.tile([128, D + 2], BF16, tag=f"pn{jc}", name=f"pn{jc}")
            nc.scalar.copy(out=t[:, 0:D], in_=f)
            nc.gpsimd.memset(t[:, D : D + 2], 1.0)
            Pf.append(f)
            Pn.append(t)
        return Pf, Pn

    def phase_pt(b, Pn):
        # P^T via paired single-pass bf16 PE transposes; PcT chunks emitted
        # right after each evacuation so the first scores matmul unblocks
        # after 2 of 4 j2 rounds instead of after the whole phase
        PT = [ptp.tile([128, PLEN], BF16, tag=f"pt{dc}", name=f"PT{dc}")
              for dc in range(ND)]
        PcT = [ptp.tile([128, PLEN], BF16, tag=f"pct{dc}", name=f"PcT{dc}")
               for dc in range(ND)]
        for j2 in range(NJ // 2):
            for dc in range(ND):
                pst = ps_t2.tile([128, 260], BF16, tag="pst", name="pst")
                nc.tensor.transpose(pst[:, 0:128], Pn[2 * j2][:, ts(dc, 128)],
                                    ident_b)
                nc.tensor.transpose(pst[:, 128:256], Pn[2 * j2 + 1][:, ts(dc, 128)],
                                    ident_b)
                nc.vector.tensor_copy(out=PT[dc][:, ts(j2, 256)],
                                      in_=pst[:, 0:256])
                nc.vector.tensor_scalar_mul(out=PcT[dc][:, ts(j2, 256)],
                                            in0=PT[dc][:, ts(j2, 256)],
                                            scalar1=wc_sb[dc])
        return PT, PcT

    def phase_scores(b, PT, PcT, Pn):
        # bias-free exp: exp(S + sb_j) = exp(S) * exp(sb_j), and since j is
        # the attn contraction partition, exp(sb_j) folds into a per-partition
        # scale of the [P | 1] moving operand (the denominator column scales
        # identically, so the softmax is unchanged).  sb[j] = sum_d PT[d,j]
        # wb[d] rides the scores stationaries as two 1-column matmuls per jc;
        # exp(sb) is read straight out of PSUM by the ACT.
        expST = [pexp.tile([128, PLEN], BF16, tag=f"es{jc}", name=f"expST{jc}")
                 for jc in range(NJ)]
        PnS = []
        sbp = None
        for jc in range(NJ):
            h = jc % 2
            if h == 0:
                sbt = ps_t2.tile([128, 260], BF16, tag="pst", name="psb")
                sbp = sbt[:, 256:260].bitcast(F32)
            for ic2 in range(2):
                pss = ps_big.tile([128, 512], F32, tag="big", name="pss")
                for dc in range(ND):
                    nc.tensor.matmul(pss, PT[dc][:, ts(jc, 128)],
                                     PcT[dc][:, ts(ic2, 512)],
                                     start=(dc == 0), stop=(dc == 1))
                nc.scalar.activation(out=expST[jc][:, ts(ic2, 512)], in_=pss,
                                     func=AF.Exp)
            for dc in range(ND):
                nc.tensor.matmul(sbp[:, h : h + 1], PT[dc][:, ts(jc, 128)],
                                 wb_col[dc], start=(dc == 0), stop=(dc == 1))
            if h == 1:
                es = pin.tile([128, 2], F32, tag=f"esb{jc // 2}",
                              name=f"esb{jc // 2}")
                nc.scalar.activation(out=es, in_=sbp, func=AF.Exp)
                for g in range(2):
                    t = pin.tile([128, D + 2], BF16, tag=f"pns{jc - 1 + g}",
                                 name=f"pns{jc - 1 + g}")
                    nc.vector.tensor_scalar_mul(out=t, in0=Pn[jc - 1 + g],
                                                scalar1=es[:, g : g + 1])
                    PnS.append(t)
        return expST, PnS

    def phase_attn(b, PnS, expST):
        # itr + denominator in one PE stream per i-chunk: stationary is the
        # expST i-chunk, moving is [P | 1] (258 cols); output col 256 is the
        # softmax denominator for those 128 queries (per-partition scalar).
        itn = []
        for ic in range(NJ):
            pia = ps_att.tile([128, D + 2], F32, tag="pia", name=f"pia{ic}")
            for jc in range(NJ):
                nc.tensor.matmul(pia, expST[jc][:, ts(ic, 128)], PnS[jc],
                                 start=(jc == 0), stop=(jc == NJ - 1))
            recip = pitr.tile([128, 1], F32, tag="recip", name="recip", bufs=4)
            nc.vector.reciprocal_approx_fast(out=recip, in_=pia[:, D : D + 1])
            t = pitr.tile([128, D], BF16, tag=f"itn{ic}", name=f"itn{ic}")
            nc.vector.tensor_scalar_mul(out=t, in0=pia[:, 0:D], scalar1=recip)
            itn.append(t)
        # transpose normalized itr into the [d, i] layout the MLP consumes
        itrT = [pitr.tile([128, PLEN], BF16, tag=f"it{dc}", name=f"itrT{dc}")
                for dc in range(ND)]
        for dc in range(ND):
            for i2 in range(NJ // 2):
                pst = ps_t2.tile([128, 260], BF16, tag="pst", name="pst")
                nc.tensor.transpose(pst[:, 0:128], itn[2 * i2][:, ts(dc, 128)],
                                    ident_b)
                nc.tensor.transpose(pst[:, 128:256], itn[2 * i2 + 1][:, ts(dc, 128)],
                                    ident_b)
                nc.vector.tensor_copy(out=itrT[dc][:, ts(i2, 256)],
                                      in_=pst[:, 0:256])
        return itrT

    def phase_mlp(b, PT, itrT):
        catT = [PT[0], PT[1], itrT[0], itrT[1]]
        oT = []
        for dc in range(ND):
            acts = []
            for wi in range(3):
                t = pmlp.tile([128, PLEN], BF16, tag=f"act{wi}", name=f"act{wi}")
                for pc in range(2):
                    psm = ps_big.tile([128, 512], F32, tag="big", name="psm")
                    for kc in range(4):
                        nc.tensor.matmul(
                            psm,
                            w_sb[wi][:, kc, ts(dc, 128)],
                            catT[kc][:, ts(pc, 512)],
                            start=(kc == 0), stop=(kc == 3),
                        )
                    nc.scalar.activation(out=t[:, ts(pc, 512)], in_=psm,
                                         func=AF.Tanh, bias=b_sb[wi][dc],
                                         scale=1.0 if wi == 0 else 0.5)
                acts.append(t)
            z_t, t2, t3 = acts
            # 2*out^T = (t2+1)*P^T + (t3+1)*z  (the x0.5 lives in the output
            # transpose identity), in p-halves so the output transposes can
            # start after the first half
            o = pmlp.tile([128, PLEN], BF16, tag=f"oT{dc}", name=f"oT{dc}")
            for pc in range(2):
                sl = ts(pc, 512)
                m1 = pmlp.tile([128, 512], BF16, tag="m1", name="m1", bufs=1)
                nc.vector.scalar_tensor_tensor(out=m1, in0=t2[:, sl], scalar=1.0,
                                               in1=PT[dc][:, sl],
                                               op0=ALU.add, op1=ALU.mult)
                m2 = pmlp.tile([128, 512], BF16, tag="m2", name="m2", bufs=1)
                nc.vector.scalar_tensor_tensor(out=m2, in0=t3[:, sl], scalar=1.0,
                                               in1=z_t[:, sl],
                                               op0=ALU.add, op1=ALU.mult)
                nc.vector.scalar_tensor_tensor(out=o[:, sl], in0=m2, scalar=0.0,
                                               in1=m1, op0=ALU.add, op1=ALU.add)
            oT.append(o)
        return oT

    def phase_mlp_out_tail(b, PT, itrT):
        # last batch element: no next-batch work exists to hide the
        # mlp->gating->transpose drain, so run it in p-halves -- the first
        # half's output transposes and DMAs overlap the second half's matmuls
        catT = [PT[0], PT[1], itrT[0], itrT[1]]
        oT = [pmlp.tile([128, PLEN], BF16, tag=f"oT{dc}", name=f"oT{dc}")
              for dc in range(ND)]
        for pc in range(2):
            sl = ts(pc, 512)
            for dc in range(ND):
                acts = []
                for wi in range(3):
                    psm = ps_big.tile([128, 512], F32, tag="big", name="psmh")
                    for kc in range(4):
                        nc.tensor.matmul(
                            psm,
                            w_sb[wi][:, kc, ts(dc, 128)],
                            catT[kc][:, sl],
                            start=(kc == 0), stop=(kc == 3),
                        )
                    t = pmlp.tile([128, 512], BF16, tag=f"acth{wi}",
                                  name=f"acth{wi}")
                    nc.scalar.activation(out=t, in_=psm, func=AF.Tanh,
                                         bias=b_sb[wi][dc],
                                         scale=1.0 if wi == 0 else 0.5)
                    acts.append(t)
                z_t, t2, t3 = acts
                m1 = pmlp.tile([128, 512], BF16, tag="m1", name="m1", bufs=1)
                nc.vector.scalar_tensor_tensor(out=m1, in0=t2, scalar=1.0,
                                               in1=PT[dc][:, sl],
                                               op0=ALU.add, op1=ALU.mult)
                m2 = pmlp.tile([128, 512], BF16, tag="m2", name="m2", bufs=1)
                nc.vector.scalar_tensor_tensor(out=m2, in0=t3, scalar=1.0,
                                               in1=z_t,
                                               op0=ALU.add, op1=ALU.mult)
                nc.vector.scalar_tensor_tensor(out=oT[dc][:, sl], in0=m2,
                                               scalar=0.0, in1=m1,
                                               op0=ALU.add, op1=ALU.add)
        for pc in range(2):
            for p2 in range(pc * 4, pc * 4 + 4):
                onat = pout.tile([128, D], F32, tag=f"on{p2}", name=f"onat{p2}")
                pst = ps_t2.tile([128, 260], BF16, tag="pst", name="psto")
                nc.tensor.transpose(pst[:, 0:128], oT[0][:, ts(p2, 128)],
                                    ident_b)
                nc.tensor.transpose(pst[:, 128:256], oT[1][:, ts(p2, 128)],
                                    ident_b)
                if p2 % 2 == 0:
                    nc.scalar.mul(out=onat, in_=pst[:, 0:256], mul=0.5)
                else:
                    nc.vector.tensor_scalar_mul(out=onat, in0=pst[:, 0:256],
                                                scalar1=0.5)
                eng = [nc.sync, nc.gpsimd, nc.scalar][p2 % 3]
                eng.dma_start(out=out[b, ts(p2, 128), :], in_=onat)

    def phase_out(b, oT):
        # bf16 transposes (oT holds 2*out in bf16); the x0.5 rides the
        # fp32 evacuation, split across ACT/DVE to shorten the serial chain
        for p2 in range(NJ):
            onat = pout.tile([128, D], F32, tag=f"on{p2}", name=f"onat{p2}")
            pst = ps_t2.tile([128, 260], BF16, tag="pst", name="psto")
            nc.tensor.transpose(pst[:, 0:128], oT[0][:, ts(p2, 128)], ident_b)
            nc.tensor.transpose(pst[:, 128:256], oT[1][:, ts(p2, 128)], ident_b)
            if p2 % 2 == 0:
                nc.scalar.mul(out=onat, in_=pst[:, 0:256], mul=0.5)
            else:
                nc.vector.tensor_scalar_mul(out=onat, in0=pst[:, 0:256],
                                            scalar1=0.5)
            nc.sync.dma_start(out=out[b, ts(p2, 128), :], in_=onat)

    # ---- software-pipelined emission across batch elements ----
    # PE order per iteration: attn(b) | out(b-1) | pt(b+1) | mlp(b) |
    # scores(b+1) -- the out/pt phases fill the attn->mlp dependency bubble
    # so the (in-order) PE never idles long enough for HAM to re-throttle.
    Pf, Pn = phase_load(0, split=True)
    PT, PcT = phase_pt(0, Pn)
    expST, PnS = phase_scores(0, PT, PcT, Pn)
    oT_prev = None
    for b in range(B_LOC):
        last = b + 1 == B_LOC
        if not last:
            Pf_n, Pn_n = phase_load(b + 1)
        itrT = phase_attn(b, PnS, expST)
        if oT_prev is not None:
            phase_out(b - 1, oT_prev)
        if last:
            phase_mlp_out_tail(b, PT, itrT)
            break
        PT_n, PcT_n = phase_pt(b + 1, Pn_n)
        oT_prev = phase_mlp(b, PT, itrT)
        expST, PnS = phase_scores(b + 1, PT_n, PcT_n, Pn_n)
        Pn, PT, PcT = Pn_n, PT_n, PcT_n


_NC_CACHE = {}


def _build():
    if "nc" in _NC_CACHE:
        return _NC_CACHE["nc"]
    nc = bacc.Bacc("TRN2", target_bir_lowering=False, debug=False,
                   num_devices=N_CORES)
    P_in = nc.dram_tensor("p_in", [B_LOC, PLEN, D], F32, kind="ExternalInput").ap()
    w_att = nc.dram_tensor("w_att", [3 * D], F32, kind="ExternalInput").ap()
    w_mlp = [nc.dram_tensor(f"w{i}", [2 * D, D], F32, kind="ExternalInput").ap()
             for i in (1, 2, 3)]
    b_mlp = [nc.dram_tensor(f"b{i}", [D], F32, kind="ExternalInput").ap()
             for i in (1, 2, 3)]
    out = nc.dram_tensor("out", [B_LOC, PLEN, D], F32, kind="ExternalOutput").ap()

    from contextlib import ExitStack

    with tile.TileContext(nc) as tc, ExitStack() as ctx:
        _emit(ctx, tc, P_in, w_att, w_mlp, b_mlp, out)
    nc.compile()
    _NC_CACHE["nc"] = nc
    return nc


def run(inputs, trace=False, tmpdir=None):
    nc = _build()
    P = np.ascontiguousarray(np.asarray(inputs["P"], dtype=np.float32))
    shared = {
        "w_att": np.ascontiguousarray(np.asarray(inputs["w_itr_att"], np.float32)),
        "w1": np.ascontiguousarray(np.asarray(inputs["w1"], np.float32)),
        "w2": np.ascontiguousarray(np.asarray(inputs["w2"], np.float32)),
        "w3": np.ascontiguousarray(np.asarray(inputs["w3"], np.float32)),
        "b1": np.ascontiguousarray(np.asarray(inputs["b1"], np.float32)),
        "b2": np.ascontiguousarray(np.asarray(inputs["b2"], np.float32)),
        "b3": np.ascontiguousarray(np.asarray(inputs["b3"], np.float32)),
    }
    in_maps = [
        {"p_in": P[c * B_LOC : (c + 1) * B_LOC], **shared} for c in range(N_CORES)
    ]
    res = run_bass_kernel_spmd(nc, in_maps, list(range(N_CORES)), trace=trace,
                               tmpdir=tmpdir)
    full = np.concatenate([res.results[c]["out"] for c in range(N_CORES)], axis=0)
    return full, res


def kernel(**inputs):
    full, _ = run(inputs)
    return full
